# revision 1
# baseline (speedup 1.0000x reference)
"""Grouped MoE MLP (SwiGLU) kernel for Trainium2, 8 NeuronCores.

Strategy (pure expert-parallel):
  Tokens arrive pre-sorted by expert with per-expert counts.  Expert e's
  weights and token block go to core e (one expert per core, no weight
  duplication -- weight bytes shipped to the device are the dominant cost
  for this problem).  Token blocks are zero-padded to a common T_pad so
  all cores run one SPMD program.

  Device program per core (dense SwiGLU MLP over T_pad tokens):
    GEMM1: h1^T[f, t] = sum_h W1[h, f] * x[t, h]     (h on partitions)
    SwiGLU on feature-partitioned tiles
    GEMM2: out[t, o]  = sum_f h[t, f] * W2[f, o]     (f on partitions,
           tokens become the PSUM partition dim so the output lands in
           natural [T, HIDDEN] layout -- no transposes on the way out)

  All device I/O is bf16 (inputs cast on host, output cast back to f32
  on host) to halve the bytes staged over the host<->device link.
"""

import math
from contextlib import ExitStack

import ml_dtypes
import numpy as np

P = 128
HIDDEN = 2048
INTER = 1408
GU = 2 * INTER            # 2816 = gate+up columns
KH = HIDDEN // P          # 16 k-tiles for GEMM1
KI = INTER // P           # 11 k-tiles for GEMM2 / gate-up pair blocks
NO = HIDDEN // 512        # 4 output column blocks of 512
N_CORES = 8
NT = 512                  # tokens per chunk (matmul moving free dim)

BF16 = ml_dtypes.bfloat16

_PROGRAM_CACHE: dict = {}


def _build_program(t_pad: int, nt: int):
    import concourse.mybir as mybir
    import concourse.tile as tile
    from concourse import bacc

    bf16 = mybir.dt.bfloat16
    f32 = mybir.dt.float32

    n_chunks = t_pad // nt
    nb = nt // P

    nc = bacc.Bacc(None, target_bir_lowering=False, debug=False)
    # chunk-major layouts: every DMA below moves one fully contiguous
    # DRAM block (x loads 128 KiB, w1 704 KiB, w2 512 KiB, out stores
    # 128 KiB) -- friendly to any memory system the tensors live in.
    # partition-major chunks: each chunk loads as ONE contiguous 2 MiB DMA
    xT = nc.dram_tensor("xT", [n_chunks, P, KH, nt], bf16, kind="ExternalInput")
    # w1 is column-group-major: group g < KI is gate cols [128g, 128g+128),
    # group KI+g is the matching up block -- so GEMM1 group mp only waits
    # for its own two 720 KiB blocks, not the whole 11.5 MiB tensor.
    w1 = nc.dram_tensor("w1", [2 * KI, P, KH, P], bf16, kind="ExternalInput")
    w2 = nc.dram_tensor("w2", [KI, P, HIDDEN], bf16, kind="ExternalInput")
    out = nc.dram_tensor(
        "out", [n_chunks, nb, NO, P, 512], bf16, kind="ExternalOutput"
    )

    with tile.TileContext(nc) as tc, ExitStack() as ctx:
        w1_pool = ctx.enter_context(tc.tile_pool(name="w1p", bufs=1))
        w2_pool = ctx.enter_context(tc.tile_pool(name="w2p", bufs=1))
        x_pool = ctx.enter_context(tc.tile_pool(name="xp", bufs=2))
        h_pool = ctx.enter_context(tc.tile_pool(name="hp", bufs=2))
        g_pool = ctx.enter_context(tc.tile_pool(name="gp", bufs=2))
        o_pool = ctx.enter_context(tc.tile_pool(name="op", bufs=3))
        ps1 = ctx.enter_context(tc.tile_pool(name="ps1", bufs=2, space="PSUM"))
        ps2 = ctx.enter_context(tc.tile_pool(name="ps2", bufs=2, space="PSUM"))

        # first x chunk goes ahead of the weight DMAs so the first GEMM1
        # matmul only waits for w1, not the whole weight set (HWDGE
        # queues are FIFO)
        xt0 = x_pool.tile([P, KH, nt], bf16, tag="xt")
        nc.sync.dma_start(xt0[:], xT[0])
        w1t = w1_pool.tile([P, 2 * KI, KH, P], bf16)
        # interleave gate/up group DMAs in mp order so group mp's two
        # blocks land just before its matmuls need them
        for mp in range(KI):
            nc.sync.dma_start(w1t[:, mp], w1[mp])
            nc.sync.dma_start(w1t[:, KI + mp], w1[KI + mp])
        w2t = w2_pool.tile([P, KI, HIDDEN], bf16)
        for k in range(KI):
            nc.sync.dma_start(w2t[:, k, :], w2[k])

        c_n = nt
        for ci in range(n_chunks):
            if ci == 0:
                xt = xt0
            else:
                xt = x_pool.tile([P, KH, c_n], bf16, tag="xt")
                nc.sync.dma_start(xt[:], xT[ci])
            ht = h_pool.tile([P, KI, c_n], bf16, tag="ht")
            for mp in range(KI):
                pg = ps1.tile([P, c_n], f32, tag="pg")
                pu = ps1.tile([P, c_n], f32, tag="pu")
                for k in range(KH):
                    nc.tensor.matmul(
                        pg[:],
                        w1t[:, mp, k, :],
                        xt[:, k, :],
                        start=(k == 0),
                        stop=(k == KH - 1),
                    )
                for k in range(KH):
                    nc.tensor.matmul(
                        pu[:],
                        w1t[:, KI + mp, k, :],
                        xt[:, k, :],
                        start=(k == 0),
                        stop=(k == KH - 1),
                    )
                gt = g_pool.tile([P, c_n], bf16, tag="gt")
                nc.scalar.activation(
                    gt[:], pg[:], mybir.ActivationFunctionType.Silu
                )
                nc.vector.tensor_mul(ht[:, mp, :], gt[:], pu[:])
            # GEMM2 with tokens on the PSUM partition dim: for each
            # 128-token block, out[tb, o_blk] = ht[:, :, tb].T @ w2
            for tb in range(c_n // P):
                for m in range(NO):
                    po = ps2.tile([P, 512], f32, tag="po")
                    for k in range(KI):
                        nc.tensor.matmul(
                            po[:],
                            ht[:, k, tb * P : (tb + 1) * P],
                            w2t[:, k, m * 512 : (m + 1) * 512],
                            start=(k == 0),
                            stop=(k == KI - 1),
                        )
                    om = o_pool.tile([P, 512], bf16, tag="om")
                    nc.vector.tensor_copy(om[:], po[:])
                    nc.sync.dma_start(out[ci, tb, m], om[:])
    nc.compile()
    return nc


def _get_program(t_pad: int, nt: int):
    key = (t_pad, nt)
    if key not in _PROGRAM_CACHE:
        _PROGRAM_CACHE[key] = _build_program(t_pad, nt)
    return _PROGRAM_CACHE[key]


def _pack_w1(w: np.ndarray) -> np.ndarray:
    # [HIDDEN, GU] f32 -> column-group-major [2*KI, P, KH, 128] bf16
    # (row h = 128k + p, col c = 128g + j)
    return w.reshape(KH, P, 2 * KI, P).transpose(2, 1, 0, 3).astype(BF16)


def _pack_w2(w: np.ndarray) -> np.ndarray:
    # [INTER, HIDDEN] f32 -> [KI, P, HIDDEN] bf16 (pure reshape + cast)
    return w.reshape(KI, P, HIDDEN).astype(BF16)


def _assign_experts(counts):
    """Expert e -> core (e % N_CORES); slots stack when E > N_CORES."""
    n_exp = len(counts)
    n_slots = max(1, math.ceil(n_exp / N_CORES))
    cores = []
    for r in range(N_CORES):
        slots = []
        for s in range(n_slots):
            e = s * N_CORES + r
            slots.append(e if e < n_exp else None)
        cores.append(slots)
    return cores, n_slots


def _run(
    hidden_states: np.ndarray,
    merged_gate_up_proj: np.ndarray,
    merged_down_proj: np.ndarray,
    num_tokens_per_expert: np.ndarray,
    trace: bool = False,
):
    counts = [int(c) for c in np.asarray(num_tokens_per_expert)]
    n_exp = len(counts)
    offs = np.concatenate([[0], np.cumsum(counts)]).astype(int)
    total = int(offs[-1])

    core_experts, n_slots = _assign_experts(counts)
    assert n_slots == 1, "more experts than cores not supported"
    per_core_rows = [sum(counts[e] for e in slots if e is not None)
                     for slots in core_experts]
    t_pad = max(NT, ((max(per_core_rows) + NT - 1) // NT) * NT)

    nc = _get_program(t_pad, NT)

    from concurrent.futures import ThreadPoolExecutor

    pool = ThreadPoolExecutor(8)

    # [TOTAL, HIDDEN] f32 -> bf16 -> transposed [HIDDEN, TOTAL] -> [KH, P, TOTAL]
    x_bf16 = hidden_states[:total].astype(BF16)
    xT_full = np.empty((HIDDEN, total), dtype=BF16)

    def _tr(k):
        xT_full[k * P : (k + 1) * P] = x_bf16[:, k * P : (k + 1) * P].T

    list(pool.map(_tr, range(KH)))
    xT_full = xT_full.reshape(KH, P, total)

    w1_packed = list(pool.map(
        lambda e: _pack_w1(merged_gate_up_proj[e]), range(n_exp)
    ))
    w2_packed = list(pool.map(
        lambda e: _pack_w2(merged_down_proj[e]), range(n_exp)
    ))

    n_chunks = t_pad // NT
    xT_pkt = xT_full.transpose(1, 0, 2)  # [P, KH, total] view

    def _core_x(r):
        e = core_experts[r][0]
        xT_core = np.zeros((n_chunks, P, KH, NT), dtype=BF16)
        if e is not None and counts[e]:
            cnt = counts[e]
            for ci in range(n_chunks):
                t0 = ci * NT
                n = min(NT, cnt - t0)
                if n <= 0:
                    break
                xT_core[ci, :, :, :n] = xT_pkt[
                    :, :, offs[e] + t0 : offs[e] + t0 + n
                ]
        return xT_core

    core_x = list(pool.map(_core_x, range(N_CORES)))
    pool.shutdown(wait=True)

    in_maps = []
    for r in range(N_CORES):
        e = core_experts[r][0]
        ew = e if (e is not None and e < n_exp) else 0
        in_maps.append(
            {"xT": core_x[r], "w1": w1_packed[ew], "w2": w2_packed[ew]}
        )

    res = _execute(nc, in_maps, trace)

    out = np.empty((total, HIDDEN), dtype=np.float32)

    def _unshard(r):
        e = core_experts[r][0]
        if e is None or counts[e] == 0:
            return
        cnt = counts[e]
        # [n_chunks, nb, NO, P, 512] -> [t, o] with t = (ci, tb, p),
        # o = (m, j)
        o_core = np.ascontiguousarray(
            res.results[r]["out"].transpose(0, 1, 3, 2, 4)
        ).reshape(t_pad, HIDDEN)
        out[offs[e] : offs[e] + cnt] = o_core[:cnt].astype(np.float32)

    upool = ThreadPoolExecutor(8)
    list(upool.map(_unshard, range(N_CORES)))
    upool.shutdown(wait=True)
    return out, res


def _execute(nc, in_maps, trace):
    from concourse.bass_utils import run_bass_kernel_spmd

    if not trace:
        try:
            return _execute_pjrt_dev_zeros(nc, in_maps)
        except Exception:
            pass
    # "out" and "xT" have identical byte sizes; the kernel's semaphore
    # chain guarantees every x row is consumed before the corresponding
    # out row is stored, so donating xT's device buffer to out is safe.
    # (On the axon path aliases are ignored; on the native path this
    # skips staging a zero buffer for the output.)
    return run_bass_kernel_spmd(
        nc, in_maps, list(range(N_CORES)), aliases={"out": "xT"}, trace=trace
    )


_EXEC_CACHE: dict = {}


def _build_pjrt_executor(nc):
    from concourse.bass_utils import axon_active
    import concourse.mybir as mybir
    from concourse import bass2jax
    import jax
    import jax.numpy as jnp
    from jax.sharding import Mesh, PartitionSpec, NamedSharding
    from jax.experimental.shard_map import shard_map

    if not axon_active():
        raise RuntimeError("pjrt path requires axon")
    if nc.dbg_addr is not None:
        raise RuntimeError("debug program")

    bass2jax.install_neuronx_cc_hook()

    partition_name = nc.partition_id_tensor.name if nc.partition_id_tensor else None
    in_names, out_names, out_avals = [], [], []
    for alloc in nc.m.functions[0].allocations:
        if not isinstance(alloc, mybir.MemoryLocationSet):
            continue
        name = alloc.memorylocations[0].name
        if alloc.kind == "ExternalInput":
            if name != partition_name:
                in_names.append(name)
        elif alloc.kind == "ExternalOutput":
            out_names.append(name)
            out_avals.append(
                jax.core.ShapedArray(
                    tuple(alloc.tensor_shape), mybir.dt.np(alloc.dtype)
                )
            )
    n_params = len(in_names)
    n_outs = len(out_avals)
    all_names = in_names + out_names
    if partition_name is not None:
        all_names = all_names + [partition_name]
    donate = tuple(range(n_params, n_params + n_outs))

    def _body(*args):
        operands = list(args)
        if partition_name is not None:
            operands.append(bass2jax.partition_id_tensor())
        outs = bass2jax._bass_exec_p.bind(
            *operands,
            out_avals=tuple(out_avals),
            in_names=tuple(all_names),
            out_names=tuple(out_names),
            lowering_input_output_aliases=(),
            sim_require_finite=True,
            sim_require_nnan=True,
            nc=nc,
        )
        return tuple(outs)

    devices = jax.devices()[:N_CORES]
    assert len(devices) == N_CORES
    mesh = Mesh(np.asarray(devices), ("core",))
    in_specs = (PartitionSpec("core"),) * (n_params + n_outs)
    out_specs = (PartitionSpec("core"),) * n_outs
    sharded = jax.jit(
        shard_map(
            _body, mesh=mesh, in_specs=in_specs, out_specs=out_specs,
            check_rep=False,
        ),
        donate_argnums=donate,
        keep_unused=True,
    )
    zsharding = NamedSharding(mesh, PartitionSpec("core"))
    zero_fns = [
        jax.jit(
            lambda s=av.shape, d=av.dtype: jnp.zeros(
                (N_CORES * s[0], *s[1:]), d
            ),
            out_shardings=zsharding,
        )
        for av in out_avals
    ]
    return {
        "sharded": sharded,
        "zero_fns": zero_fns,
        "in_names": in_names,
        "out_names": out_names,
        "out_avals": out_avals,
    }


def _execute_pjrt_dev_zeros(nc, in_maps):
    """run_bass_via_pjrt equivalent, but the donated zero output buffers
    are created on-device (jnp.zeros under jit) instead of being staged
    from host numpy -- saves shipping one full output-sized array of
    zeros per core over the host->device link."""
    from concourse.bass_utils import BassKernelResults

    key = id(nc)
    if key not in _EXEC_CACHE:
        _EXEC_CACHE[key] = _build_pjrt_executor(nc)
    ex = _EXEC_CACHE[key]

    concat_in = [
        np.concatenate([np.asarray(m[name]) for m in in_maps], axis=0)
        for name in ex["in_names"]
    ]
    dev_zeros = [fn() for fn in ex["zero_fns"]]
    out_arrs = ex["sharded"](*concat_in, *dev_zeros)
    out_avals = ex["out_avals"]
    results = [
        {
            name: np.asarray(out_arrs[i]).reshape(
                N_CORES, *out_avals[i].shape
            )[c]
            for i, name in enumerate(ex["out_names"])
        }
        for c in range(N_CORES)
    ]
    return BassKernelResults(
        results=results,
        instructions_and_trace=None,
        profile_json=None,
        exec_time_ns=None,
    )


def kernel(**inputs) -> np.ndarray:
    return _run(**inputs, trace=False)[0]


def run_traced(**inputs):
    return _run(**inputs, trace=True)



# revision 3
# speedup vs baseline: 1.4276x; 1.4276x over previous
"""Grouped MoE MLP (SwiGLU) kernel for Trainium2, 8 NeuronCores.

Strategy (load-balanced expert-parallel):
  The per-expert token counts are ragged (max 3072 vs mean 2048), so the
  baseline one-expert-per-core split leaves the hot core with 1.5x the
  average work -- and the trace shows TensorE is 96% busy, i.e. the
  kernel is at the matmul roofline for whatever token count the hot
  core carries.  The only lever is balance.

  Tokens are split into NT=256-token chunks (every chunk belongs to one
  expert; tokens arrive pre-sorted by expert).  The 64 chunks are packed
  into 8 cores x fixed per-core "slots" (e.g. sizes (3,3,2) chunks): one
  slot processes chunks of a single expert, so each core runs exactly
  sum(S) chunks = 2048 tokens.  A small backtracking packer finds a
  zero-waste structure for the given counts (for the reference counts
  the (3,3,2) packing is exact).

  Per-slot weights are streamed from HBM: GEMM1 weights at (gate,up)
  column-group granularity through a small SBUF ring (a group is dead
  once its GEMM1 finishes, so the full 11.5 MiB never sits in SBUF),
  GEMM2 weights one 5.8 MiB block per slot, double-use-free.  DMA per
  core totals ~69 MiB against ~190 us of bandwidth -- well hidden under
  ~460 us of matmul work.

  Device program per slot (dense SwiGLU over the slot's chunks):
    GEMM1 group-major: for mp in 0..10: stream w1[mp] (gate+up), then
      for each chunk: accumulate 16 k-tiles into PSUM for gate and up,
      SiLU (ACT) * up (DVE) -> h[:, mp, :] in SBUF (bf16)
    GEMM2 per chunk: tokens on the PSUM partition dim, out lands in
      natural [T, HIDDEN] layout.

  All device I/O is bf16 (cast on host) to halve staged bytes.
"""

import math
from contextlib import ExitStack

import ml_dtypes
import numpy as np

P = 128
HIDDEN = 2048
INTER = 1408
GU = 2 * INTER            # 2816 gate+up columns
KH = HIDDEN // P          # 16 k-tiles for GEMM1
KI = INTER // P           # 11 feature groups / GEMM2 k-tiles
NO = HIDDEN // 512        # 4 output column blocks of 512
N_CORES = 8
NT = 256                  # tokens per chunk
TB = NT // P              # 128-token blocks per chunk (2)

BF16 = ml_dtypes.bfloat16

_PROGRAM_CACHE: dict = {}


# --------------------------------------------------------------------------
# chunk -> slot packing
# --------------------------------------------------------------------------

def _structures(c):
    """All descending partitions of c into 1..4 parts of size <= 6."""
    out = []

    def rec(rem, maxp, cur):
        if rem == 0:
            out.append(tuple(cur))
            return
        if len(cur) == 4:
            return
        for p in range(min(maxp, rem), 0, -1):
            rec(rem - p, p, cur + [p])

    rec(c, min(c, 6), [])
    out.sort(key=lambda s: (len(s), -min(s)))
    return out


def _pack(m, S):
    """Pack expert chunk-counts m into 8 copies of slot structure S.

    Returns {(core, slot_idx): (expert, n_chunks)} or None.  A slot holds
    chunks of a single expert and may be partially filled (padding)."""
    slots = []
    for si, sz in enumerate(S):
        for core in range(N_CORES):
            slots.append((sz, core, si))
    slots.sort(key=lambda t: -t[0])
    rem = list(m)
    assign = {}
    nodes = [0]

    def feasible(i):
        caps = [s[0] for s in slots[i:]]
        need = [r for r in rem if r > 0]
        if not need:
            return True
        if not caps or sum(caps) < sum(need):
            return False
        mx = max(caps)
        return sum((r + mx - 1) // mx for r in need) <= len(caps)

    def rec(i):
        nodes[0] += 1
        if nodes[0] > 300000:
            return False
        if all(r == 0 for r in rem):
            return True
        if i == len(slots) or not feasible(i):
            return False
        sz, core, si = slots[i]
        cands = sorted(
            (e for e in range(len(rem)) if rem[e] > 0),
            key=lambda e: (rem[e] != sz, -rem[e]),
        )
        tried = set()
        for e in cands:
            amt = min(rem[e], sz)
            if amt in tried:
                continue
            tried.add(amt)
            rem[e] -= amt
            assign[(core, si)] = (e, amt)
            if rec(i + 1):
                return True
            del assign[(core, si)]
            rem[e] += amt
        return rec(i + 1)  # leave this slot empty

    return assign if rec(0) else None


def _plan(counts):
    """-> (S, cores) where cores[r] = [(expert|None, echunk0, n_real), ...]
    one entry per slot of S."""
    m = [(c + NT - 1) // NT for c in counts]
    total = sum(m)
    base = (total + N_CORES - 1) // N_CORES
    for c in range(base, base + 9):
        for S in _structures(c):
            asg = _pack(list(m), S)
            if asg is None:
                continue
            # hand out chunk ranges per expert in deterministic slot order
            slots = []
            for si, sz in enumerate(S):
                for core in range(N_CORES):
                    slots.append((sz, core, si))
            slots.sort(key=lambda t: -t[0])
            nxt = [0] * len(m)
            cores = [[None] * len(S) for _ in range(N_CORES)]
            for sz, core, si in slots:
                ent = asg.get((core, si))
                if ent is None:
                    continue
                e, amt = ent
                cores[core][si] = (e, nxt[e], amt)
                nxt[e] += amt
            return S, cores
    raise RuntimeError(f"no packing found for counts {counts}")


# --------------------------------------------------------------------------
# device program
# --------------------------------------------------------------------------

def _build_program(S):
    import concourse.mybir as mybir
    import concourse.tile as tile
    from concourse import bacc

    bf16 = mybir.dt.bfloat16
    f32 = mybir.dt.float32

    n_slots = len(S)
    n_chunks = sum(S)

    nc = bacc.Bacc(None, target_bir_lowering=False, debug=False)
    # x: chunk-major, hidden on partitions; each chunk one contiguous 1 MiB DMA
    xT = nc.dram_tensor("xT", [n_chunks, P, KH, NT], bf16, kind="ExternalInput")
    # w1: per (slot, group): [P, 2(gate/up), KH, P] contiguous 1 MiB blocks
    w1 = nc.dram_tensor(
        "w1", [n_slots, KI, P, 2, KH, P], bf16, kind="ExternalInput"
    )
    # w2: per slot: [P, KI, HIDDEN] contiguous 5.5 MiB block
    w2 = nc.dram_tensor("w2", [n_slots, P, KI, HIDDEN], bf16, kind="ExternalInput")
    out = nc.dram_tensor(
        "out", [n_chunks, TB, NO, P, 512], bf16, kind="ExternalOutput"
    )

    with tile.TileContext(nc) as tc, ExitStack() as ctx:
        w1_pool = ctx.enter_context(tc.tile_pool(name="w1p", bufs=6))
        w2_pool = ctx.enter_context(tc.tile_pool(name="w2p", bufs=1))
        x_pool = ctx.enter_context(tc.tile_pool(name="xp", bufs=6))
        h_pool = ctx.enter_context(tc.tile_pool(name="hp", bufs=6))
        g_pool = ctx.enter_context(tc.tile_pool(name="gp", bufs=3))
        o_pool = ctx.enter_context(tc.tile_pool(name="op", bufs=4))
        ps1 = ctx.enter_context(tc.tile_pool(name="ps1", bufs=2, space="PSUM"))
        ps2 = ctx.enter_context(tc.tile_pool(name="ps2", bufs=3, space="PSUM"))

        g0 = 0
        for si, sz in enumerate(S):
            # ---- DMA emission for this slot ----
            # order on the sync ring: x chunk0, w1 group0, rest of x, then
            # w1 groups 1..10 (first matmul gates on ~2 MiB only).
            # w2 + out stores ride the scalar ring so they never head-of-
            # line block the sync ring.
            xts = []
            w1ts = []
            for j in range(sz):
                t = x_pool.tile([P, KH, NT], bf16, tag="xt")
                nc.sync.dma_start(t[:], xT[g0 + j])
                xts.append(t)
                if j == 0:
                    t0 = w1_pool.tile([P, 2, KH, P], bf16, tag="w1g")
                    nc.sync.dma_start(t0[:], w1[si, 0])
                    w1ts.append(t0)
            for mp in range(1, KI):
                t = w1_pool.tile([P, 2, KH, P], bf16, tag="w1g")
                nc.sync.dma_start(t[:], w1[si, mp])
                w1ts.append(t)
            w2t = w2_pool.tile([P, KI, HIDDEN], bf16, tag="w2t")
            nc.scalar.dma_start(w2t[:], w2[si])

            # ---- GEMM1 (group-major over the slot's chunks) ----
            hts = []
            for _ in range(sz):
                ht = h_pool.tile([P, KI, NT], bf16, tag="ht")
                hts.append(ht)
            for mp in range(KI):
                w1t = w1ts[mp]
                for j in range(sz):
                    pg = ps1.tile([P, NT], f32, tag="pg")
                    pu = ps1.tile([P, NT], f32, tag="pu")
                    for k in range(KH):
                        nc.tensor.matmul(
                            pg[:],
                            w1t[:, 0, k],
                            xts[j][:, k],
                            start=(k == 0),
                            stop=(k == KH - 1),
                        )
                    for k in range(KH):
                        nc.tensor.matmul(
                            pu[:],
                            w1t[:, 1, k],
                            xts[j][:, k],
                            start=(k == 0),
                            stop=(k == KH - 1),
                        )
                    gt = g_pool.tile([P, NT], bf16, tag="gt")
                    nc.scalar.activation(
                        gt[:], pg[:], mybir.ActivationFunctionType.Silu
                    )
                    nc.vector.tensor_mul(hts[j][:, mp], gt[:], pu[:])

            # ---- GEMM2 (tokens on PSUM partitions) ----
            for j in range(sz):
                for tb in range(TB):
                    for m in range(NO):
                        po = ps2.tile([P, 512], f32, tag="po")
                        for k in range(KI):
                            nc.tensor.matmul(
                                po[:],
                                hts[j][:, k, tb * P : (tb + 1) * P],
                                w2t[:, k, m * 512 : (m + 1) * 512],
                                start=(k == 0),
                                stop=(k == KI - 1),
                            )
                        om = o_pool.tile([P, 512], bf16, tag="om")
                        nc.vector.tensor_copy(om[:], po[:])
                        nc.scalar.dma_start(out[g0 + j, tb, m], om[:])
            g0 += sz
    nc.compile()
    return nc


def _get_program(S):
    if S not in _PROGRAM_CACHE:
        _PROGRAM_CACHE[S] = _build_program(S)
    return _PROGRAM_CACHE[S]


# --------------------------------------------------------------------------
# host-side pack / unpack
# --------------------------------------------------------------------------

def _pack_w1(w: np.ndarray) -> np.ndarray:
    # [HIDDEN, GU] f32 -> [KI, P, 2, KH, P] bf16  (h = 128k + p, c = g*INTER
    # + 128*mp + j)
    return np.ascontiguousarray(
        w.reshape(KH, P, 2, KI, P).transpose(3, 1, 2, 0, 4)
    ).astype(BF16)


def _pack_w2(w: np.ndarray) -> np.ndarray:
    # [INTER, HIDDEN] f32 -> [P, KI, HIDDEN] bf16
    return np.ascontiguousarray(
        w.reshape(KI, P, HIDDEN).transpose(1, 0, 2)
    ).astype(BF16)


def _run(
    hidden_states: np.ndarray,
    merged_gate_up_proj: np.ndarray,
    merged_down_proj: np.ndarray,
    num_tokens_per_expert: np.ndarray,
    trace: bool = False,
):
    counts = [int(c) for c in np.asarray(num_tokens_per_expert)]
    n_exp = len(counts)
    offs = np.concatenate([[0], np.cumsum(counts)]).astype(int)
    total = int(offs[-1])

    S, cores = _plan(counts)
    n_slots = len(S)
    n_chunks = sum(S)
    slot_base = np.concatenate([[0], np.cumsum(S)]).astype(int)

    nc = _get_program(S)

    from concurrent.futures import ThreadPoolExecutor

    pool = ThreadPoolExecutor(8)

    # [TOTAL, HIDDEN] f32 -> bf16 -> [P, KH, total] transposed view source
    x_bf16 = hidden_states[:total].astype(BF16)
    xT_full = np.empty((HIDDEN, total), dtype=BF16)

    def _tr(k):
        xT_full[k * P : (k + 1) * P] = x_bf16[:, k * P : (k + 1) * P].T

    list(pool.map(_tr, range(KH)))
    xT_pkt = xT_full.reshape(KH, P, total).transpose(1, 0, 2)  # [P, KH, total]

    w1_packed = list(pool.map(
        lambda e: _pack_w1(merged_gate_up_proj[e]), range(n_exp)
    ))
    w2_packed = list(pool.map(
        lambda e: _pack_w2(merged_down_proj[e]), range(n_exp)
    ))

    def _core_inputs(r):
        xc = np.zeros((n_chunks, P, KH, NT), dtype=BF16)
        w1c = np.empty((n_slots, KI, P, 2, KH, P), dtype=BF16)
        w2c = np.empty((n_slots, P, KI, HIDDEN), dtype=BF16)
        for si in range(n_slots):
            ent = cores[r][si]
            e = ent[0] if ent is not None else 0
            w1c[si] = w1_packed[e]
            w2c[si] = w2_packed[e]
            if ent is None:
                continue
            e, k0, amt = ent
            for j in range(amt):
                t0 = (k0 + j) * NT
                n = min(NT, counts[e] - t0)
                if n <= 0:
                    break
                xc[slot_base[si] + j, :, :, :n] = xT_pkt[
                    :, :, offs[e] + t0 : offs[e] + t0 + n
                ]
        return {"xT": xc, "w1": w1c, "w2": w2c}

    in_maps = list(pool.map(_core_inputs, range(N_CORES)))
    pool.shutdown(wait=True)

    res = _execute(nc, in_maps, trace)

    out = np.empty((total, HIDDEN), dtype=np.float32)

    def _unshard(r):
        o = res.results[r]["out"]  # [n_chunks, TB, NO, P, 512] bf16
        o = o.transpose(0, 1, 3, 2, 4).reshape(n_chunks, NT, HIDDEN)
        for si in range(n_slots):
            ent = cores[r][si]
            if ent is None:
                continue
            e, k0, amt = ent
            for j in range(amt):
                t0 = (k0 + j) * NT
                n = min(NT, counts[e] - t0)
                if n <= 0:
                    break
                out[offs[e] + t0 : offs[e] + t0 + n] = o[
                    slot_base[si] + j, :n
                ].astype(np.float32)

    upool = ThreadPoolExecutor(8)
    list(upool.map(_unshard, range(N_CORES)))
    upool.shutdown(wait=True)
    return out, res


# --------------------------------------------------------------------------
# execution (pjrt fast path with on-device zero outputs, axon fallback)
# --------------------------------------------------------------------------

def _execute(nc, in_maps, trace):
    from concourse.bass_utils import run_bass_kernel_spmd

    if not trace:
        try:
            return _execute_pjrt_dev_zeros(nc, in_maps)
        except Exception:
            pass
    return run_bass_kernel_spmd(
        nc, in_maps, list(range(N_CORES)), trace=trace
    )


_EXEC_CACHE: dict = {}


def _build_pjrt_executor(nc):
    from concourse.bass_utils import axon_active
    import concourse.mybir as mybir
    from concourse import bass2jax
    import jax
    import jax.numpy as jnp
    from jax.sharding import Mesh, PartitionSpec, NamedSharding
    from jax.experimental.shard_map import shard_map

    if not axon_active():
        raise RuntimeError("pjrt path requires axon")
    if nc.dbg_addr is not None:
        raise RuntimeError("debug program")

    bass2jax.install_neuronx_cc_hook()

    partition_name = nc.partition_id_tensor.name if nc.partition_id_tensor else None
    in_names, out_names, out_avals = [], [], []
    for alloc in nc.m.functions[0].allocations:
        if not isinstance(alloc, mybir.MemoryLocationSet):
            continue
        name = alloc.memorylocations[0].name
        if alloc.kind == "ExternalInput":
            if name != partition_name:
                in_names.append(name)
        elif alloc.kind == "ExternalOutput":
            out_names.append(name)
            out_avals.append(
                jax.core.ShapedArray(
                    tuple(alloc.tensor_shape), mybir.dt.np(alloc.dtype)
                )
            )
    n_params = len(in_names)
    n_outs = len(out_avals)
    all_names = in_names + out_names
    if partition_name is not None:
        all_names = all_names + [partition_name]
    donate = tuple(range(n_params, n_params + n_outs))

    def _body(*args):
        operands = list(args)
        if partition_name is not None:
            operands.append(bass2jax.partition_id_tensor())
        outs = bass2jax._bass_exec_p.bind(
            *operands,
            out_avals=tuple(out_avals),
            in_names=tuple(all_names),
            out_names=tuple(out_names),
            lowering_input_output_aliases=(),
            sim_require_finite=True,
            sim_require_nnan=True,
            nc=nc,
        )
        return tuple(outs)

    devices = jax.devices()[:N_CORES]
    assert len(devices) == N_CORES
    mesh = Mesh(np.asarray(devices), ("core",))
    in_specs = (PartitionSpec("core"),) * (n_params + n_outs)
    out_specs = (PartitionSpec("core"),) * n_outs
    sharded = jax.jit(
        shard_map(
            _body, mesh=mesh, in_specs=in_specs, out_specs=out_specs,
            check_rep=False,
        ),
        donate_argnums=donate,
        keep_unused=True,
    )
    zsharding = NamedSharding(mesh, PartitionSpec("core"))
    zero_fns = [
        jax.jit(
            lambda s=av.shape, d=av.dtype: jnp.zeros(
                (N_CORES * s[0], *s[1:]), d
            ),
            out_shardings=zsharding,
        )
        for av in out_avals
    ]
    return {
        "sharded": sharded,
        "zero_fns": zero_fns,
        "in_names": in_names,
        "out_names": out_names,
        "out_avals": out_avals,
    }


def _execute_pjrt_dev_zeros(nc, in_maps):
    """run_bass_via_pjrt equivalent with donated zero output buffers created
    on-device instead of staged from host numpy."""
    from concourse.bass_utils import BassKernelResults

    key = id(nc)
    if key not in _EXEC_CACHE:
        _EXEC_CACHE[key] = _build_pjrt_executor(nc)
    ex = _EXEC_CACHE[key]

    concat_in = [
        np.concatenate([np.asarray(m[name]) for m in in_maps], axis=0)
        for name in ex["in_names"]
    ]
    dev_zeros = [fn() for fn in ex["zero_fns"]]
    out_arrs = ex["sharded"](*concat_in, *dev_zeros)
    out_avals = ex["out_avals"]
    results = [
        {
            name: np.asarray(out_arrs[i]).reshape(
                N_CORES, *out_avals[i].shape
            )[c]
            for i, name in enumerate(ex["out_names"])
        }
        for c in range(N_CORES)
    ]
    return BassKernelResults(
        results=results,
        instructions_and_trace=None,
        profile_json=None,
        exec_time_ns=None,
    )


def kernel(**inputs) -> np.ndarray:
    return _run(**inputs, trace=False)[0]


def run_traced(**inputs):
    return _run(**inputs, trace=True)
